# revision 26
# baseline (speedup 1.0000x reference)
"""ConvKAN Trainium2 kernel (v8: bc-half-blocked planes, all-halves waves).

Decomposition (validated vs reference):
  out[(b, cin, kh, kw, q), oc] =
      sum_{func, jh, jw} Wf[oc, func, jh*48+jw] * F_func(x_pad[b, cin, 12q+jh+kh, jw+kw])
  where F_0 = silu and F_{1+g}(v) = spline cubes 4*r1^3 - r2^3 with
  t = |2.5 v + 3.5 - g|, r2 = max(2-t, 0), r1 = max(1-t, 0)
  (weights carry the -1/6 normalization).

Sharding: input channels cin split 8 ways (8 per core); core k produces
output rows [288k, 288k+288) of (B, 2304, OUT_C).

Scheduling (measured ~107-109 us vs 153 us baseline):
- every function plane (and its x input) is stored bc-half-blocked
  [p, (x:2, h:50, b':64)], so the elementwise chains run on flat
  contiguous chunks (no strided-AP penalty) and every matmul wave runs
  as N=256 bc-halves with a contiguous rhs inner dim; a wave can start
  as soon as half a plane is ready.  Half-waves pace at the pure
  streaming rate (~109 ns) because kh-inner triples share lhsT.
- PSUM keeps the (q, b) layout; half waves write b-slices.  start=True
  only on the first matmul touching each bank, one stop on the last
  (per-element has_written semantics make partial-width accumulation
  sound).
- inputs stream on one HWDGE queue in consumption order: xs[h0] ->
  wq[silu] -> xs[h1] -> xp0 -> wq[p0,j<6] -> xp1[h0] -> wq[p0,j>=6] ->
  xp1[h1] -> wq[p1,j<6] -> xp2[h0] -> wq[p1,j>=6] -> xp2[h1] -> wq[p2].
- spline chains are emitted stage-major within each bc-half; square ops
  are split between ACT and DVE per knobs (front-loaded for pass-0 h0,
  the only remaining plane stall, ~2.5 us).
- zero-tile warmup matmuls keep the PE busy from engine init (HAM clock
  gate opens ~4 us before the first real wave), and zero-weight filler
  matmuls bridge the pass-0 plane wait so the clock never re-throttles.
- group 8 runs right after group 0's bank drains; outputs are fp16.
"""

from contextlib import ExitStack

import numpy as np

import concourse.bass as bass
import concourse.bacc as bacc
import concourse.tile as tile
from concourse import mybir
from concourse.alu_op_type import AluOpType
from concourse.bass_utils import run_bass_kernel_spmd

AF = mybir.ActivationFunctionType
DT = mybir.dt

B, C, H, W = 16, 64, 48, 48
OUT_C = 128
NCORES = 8
CLOC = C // NCORES          # 8 input channels per core
BC = B * CLOC               # 128 (b, c) pairs per core
HP = 50                     # padded height
FREE = HP * BC              # 6400
NSP = 3                     # spline passes
NTILE = 3 * 6 + NSP * 3 * 12  # 126 lhsT tiles: (silu kw jj) + (pass kw jh)
FCH = 4                     # chunks per activation pass (1600 cols each)
RUN_KWARGS = {}
LAST_EXEC_NS = None
N_WARMUP = 70               # HAM warm-up dummy matmuls (N=128, span the DMA wait)

# engine-assignment knobs, [pass][chunk]: square ops on ACT vs DVE
S2_ACT = ((False, False, True, True),
          (True, True, False, False),
          (True, True, False, False))
S1F_ACT = ((True, True, True, True),
           (True, True, True, True),
           (True, True, True, True))

V0 = (0.0, 0.0, -0.125, -2.875, -2.875, -0.125, 0.0, 0.0)  # slot value at x=0


def build_nc(fch: int = FCH) -> bass.Bass:
    nc = bacc.Bacc(None, target_bir_lowering=False, debug=True)
    xs = nc.declare_dram_parameter("xs", [128, FREE], DT.float16, isOutput=False)
    xp = nc.declare_dram_parameter("xp", [128, NSP * FREE], DT.float16,
                                   isOutput=False)
    wq = nc.declare_dram_parameter("wq", [128, NTILE * 128], DT.float16,
                                   isOutput=False)
    bias = nc.declare_dram_parameter("bias", [128, 8], DT.float32, isOutput=False)
    out = nc.declare_dram_parameter("out", [9, 128, 512], DT.float16, isOutput=True)

    fw = FREE // fch
    with ExitStack() as ctx:
        tc = ctx.enter_context(tile.TileContext(nc))
        wpool = ctx.enter_context(tc.tile_pool(name="w", bufs=1))
        fpool = ctx.enter_context(tc.tile_pool(name="f", bufs=3))
        psum_pool = ctx.enter_context(tc.tile_pool(name="ps", bufs=8, space="PSUM"))
        opool = ctx.enter_context(tc.tile_pool(name="o", bufs=4))

        bias_sb = wpool.tile([128, 8], DT.float32)
        nc.gpsimd.dma_start(bias_sb[:], bias[:])

        xs_sb = wpool.tile([128, FREE], DT.float16)
        wq_sb = wpool.tile([128, NTILE * 128], DT.float16)
        xp_sb = [wpool.tile([128, FREE], DT.float16, name=f"xp{c}", tag=f"xp{c}")
                 for c in range(NSP)]

        # input DMA stream in consumption order on the HWDGE queue
        def dma_chunks(dst, src_base, cs):
            for f in cs:
                nc.sync.dma_start(dst[:, f * fw:(f + 1) * fw],
                                  xp[:, src_base + f * fw:src_base + (f + 1) * fw]
                                  if src_base is not None else
                                  xs[:, f * fw:(f + 1) * fw])

        def dma_wq(t0, t1):
            nc.sync.dma_start(wq_sb[:, t0 * 128:t1 * 128], wq[:, t0 * 128:t1 * 128])

        dma_chunks(xs_sb, None, (0, 1))
        dma_wq(0, 18)                      # silu tiles
        dma_chunks(xs_sb, None, (2, 3))
        dma_chunks(xp_sb[0], 0, range(fch))
        dma_wq(18, 36)                     # p0 j0-5
        dma_chunks(xp_sb[1], FREE, (0, 1))
        dma_wq(36, 54)                     # p0 j6-11
        dma_chunks(xp_sb[1], FREE, (2, 3))
        dma_wq(54, 72)                     # p1 j0-5
        dma_chunks(xp_sb[2], 2 * FREE, (0, 1))
        dma_wq(72, 90)                     # p1 j6-11
        dma_chunks(xp_sb[2], 2 * FREE, (2, 3))
        dma_wq(90, 126)                    # p2

        ts_s = wpool.tile([128, FREE], DT.float16, name="tsS", tag="tsS")
        ts_t = [wpool.tile([128, FREE], DT.float16, name=f"ts{c}", tag=f"ts{c}")
                for c in range(NSP)]

        groups = [(kh, kw) for kh in range(3) for kw in range(3)]
        ps_tiles = {}
        for g in groups[:8]:
            ps_tiles[g] = psum_pool.tile([128, 512], DT.float32,
                                         name=f"ps_{g[0]}{g[1]}", tag="ps")
        zt = wpool.tile([128, 128], DT.float16, name="zt", tag="zt")
        nc.vector.memset(zt[:], 0.0)
        # HAM warm-up into group-7's bank (cleared by its first start=True mm);
        # zero-tile operands need no DMA, so the PE is busy (and the clock
        # gate open) from right after engine init until the real waves start
        warm = ps_tiles[groups[7]][:, 0:128]
        for _ in range(N_WARMUP):
            nc.tensor.matmul(warm, zt[:], zt[:], start=True, stop=False)

        # silu chain: one ACT op per chunk (chunks 0,1 = half 0)
        for f in range(fch):
            sl = slice(f * fw, (f + 1) * fw)
            nc.scalar.activation(ts_s[:, sl], xs_sb[:, sl], AF.Silu)

        # spline chains (flat chunks; stage-major within each bc-half so the
        # DVE pipeline is not serialized behind one chunk's whole chain)
        for c in range(NSP):
            bias_ap = bias_sb[:, c:c + 1]
            for half in (0, 1):
                chunks = (2 * half, 2 * half + 1)
                tl = {}
                for f in chunks:
                    sl = slice(f * fw, (f + 1) * fw)
                    t = fpool.tile([128, fw], DT.float16, name="t", tag="t")
                    nc.scalar.activation(t[:], xp_sb[c][:, sl], AF.Abs,
                                         bias=bias_ap, scale=2.5)
                    tl[f] = {"t": t}
                for f in chunks:
                    nr2 = fpool.tile([128, fw], DT.float16, name="nr2", tag="nr2")
                    nc.vector.tensor_scalar(nr2[:], tl[f]["t"][:], 2.0, 0.0,
                                            op0=AluOpType.subtract,
                                            op1=AluOpType.min)
                    nr1 = fpool.tile([128, fw], DT.float16, name="nr1", tag="nr1")
                    nc.vector.tensor_scalar(nr1[:], tl[f]["t"][:], 1.0, 0.0,
                                            op0=AluOpType.subtract,
                                            op1=AluOpType.min)
                    tl[f]["nr2"], tl[f]["nr1"] = nr2, nr1
                for f in chunks:
                    s2 = fpool.tile([128, fw], DT.float16, name="s2", tag="s2")
                    if S2_ACT[c][f]:
                        nc.scalar.activation(s2[:], tl[f]["nr2"][:], AF.Square)
                    else:
                        nc.vector.tensor_tensor(s2[:], tl[f]["nr2"][:],
                                                tl[f]["nr2"][:], op=AluOpType.mult)
                    tl[f]["s2"] = s2
                for f in chunks:
                    s1f = fpool.tile([128, fw], DT.float16, name="s1f", tag="s1f")
                    if S1F_ACT[c][f]:
                        nc.scalar.activation(s1f[:], tl[f]["nr1"][:], AF.Square,
                                             scale=2.0)
                    else:
                        nc.vector.scalar_tensor_tensor(
                            s1f[:], tl[f]["nr1"][:], 4.0, tl[f]["nr1"][:],
                            op0=AluOpType.mult, op1=AluOpType.mult)
                    tl[f]["s1f"] = s1f
                for f in chunks:
                    c2n = fpool.tile([128, fw], DT.float16, name="c2n", tag="c2n")
                    nc.vector.tensor_tensor(c2n[:], tl[f]["s2"][:],
                                            tl[f]["nr2"][:], op=AluOpType.mult)
                    tl[f]["c2n"] = c2n
                for f in chunks:
                    cn1 = fpool.tile([128, fw], DT.float16, name="cn1", tag="cn1")
                    nc.vector.tensor_tensor(cn1[:], tl[f]["s1f"][:],
                                            tl[f]["nr1"][:], op=AluOpType.mult)
                    tl[f]["cn1"] = cn1
                for f in chunks:
                    sl = slice(f * fw, (f + 1) * fw)
                    nc.vector.tensor_tensor(ts_t[c][:, sl], tl[f]["c2n"][:],
                                            tl[f]["cn1"][:], op=AluOpType.subtract)

        def emit_mm(g, seq, x, start=False, stop=False):
            kh, kw = g
            kind, c, j = seq
            if kind == "S":
                idx = kw * 6 + j
                src = ts_s
            else:
                idx = 18 + c * 36 + j * 3 + kw     # j-major pass tiles
                src = ts_t[c]
            lhsT = wq_sb[:, idx * 128:(idx + 1) * 128]
            h0 = kh + j
            ps = ps_tiles[g][:].rearrange("p (q b) -> p q b", b=BC)
            rhs = src[:].rearrange("p (x h b) -> p x h b", x=2, b=64)[
                :, x, h0:h0 + 37:12, :]
            out_ap = ps[:, :, 64 * x:64 * (x + 1)]
            nc.tensor.matmul(out_ap, lhsT, rhs, start=start, stop=stop)

        def drain(g):
            ob = opool.tile([128, 512], DT.float16)
            # adds the constant contribution of the removed w_pad 0/49 slots;
            # on DVE (idle, empty queue at the tail) so the final drain
            # dispatches right after its group's stop matmul.  Split in
            # halves so the first out-DMA overlaps the second half's drain.
            for h0, h1 in ((0, 256), (256, 512)):
                nc.vector.tensor_scalar(ob[:, h0:h1], ps_tiles[g][:, h0:h1],
                                        bias_sb[:, 4 + g[1]:5 + g[1]], None,
                                        op0=AluOpType.add)
                nc.sync.dma_start(out[g[0] * 3 + g[1]][:, h0:h1], ob[:, h0:h1])

        wave = groups[:8]
        silu_seqs = [("S", 0, j) for j in range(6)]
        pass_seqs = [[("P", c, j) for j in range(12)] for c in range(NSP)]

        def kworder(include_g8=False):
            gs = groups[:9] if include_g8 else groups[:8]
            return sorted(gs, key=lambda g: (g[1], g[0]))  # kw major, kh inner

        # per bc-half waves; kh-inner triples share lhsT
        for x in (0, 1):
            for j, s in enumerate(silu_seqs):
                for g in kworder():
                    emit_mm(g, s, x, start=(x == 0 and j == 0))
        # zero-weight fillers keep the PE (and its HAM clock state) busy
        # through the pass-0 plane wait; they add 0 to a live bank
        for _ in range(30):
            nc.tensor.matmul(ps_tiles[groups[0]][:, 0:256], zt[:],
                             ts_s[:, 0:256], start=False, stop=False)
        for c in (0, 1):
            for x in (0, 1):
                for s in pass_seqs[c]:
                    for g in kworder():
                        emit_mm(g, s, x)
        # pass 2: group 0 first, drain it, then g8's full run, then the rest
        for x in (0, 1):
            for i, s in enumerate(pass_seqs[2]):
                emit_mm(wave[0], s, x, stop=(x == 1 and i == 11))
        drain(wave[0])
        g8 = groups[8]
        ps_tiles[g8] = psum_pool.tile([128, 512], DT.float32, name="ps_22",
                                      tag="ps")
        g8_seqs = [(s, x) for s in silu_seqs + pass_seqs[0] + pass_seqs[1]
                   + pass_seqs[2] for x in (0, 1)]
        for i, (s, x) in enumerate(g8_seqs):
            emit_mm(g8, s, x, start=(i == 0), stop=(i == len(g8_seqs) - 1))
        drain(g8)
        for g in [g for g in kworder() if g != wave[0]]:
            for x in (0, 1):
                for i, s in enumerate(pass_seqs[2]):
                    emit_mm(g, s, x, stop=(x == 1 and i == 11))
            drain(g)
    nc.compile()
    return nc


def _prep_weights(base_weight, spline_weight, spline_scaler):
    # Wf[oc, func, jj]: func 0 = silu weights, 1+g = scaled spline / -6
    wf = np.empty((OUT_C, 9, 576), dtype=np.float64)
    wf[:, 0, :] = base_weight
    wf[:, 1:, :] = np.moveaxis(
        spline_weight.astype(np.float64)
        * spline_scaler.astype(np.float64)[..., None] / -6.0, -1, 1)
    w4 = wf.reshape(OUT_C, 9, 12, 48)
    wq = np.zeros((128, NTILE, OUT_C), dtype=np.float64)
    for kw in range(3):
        for jj in range(6):  # silu tiles
            idx = kw * 6 + jj
            for p in range(96):
                s, wp = p // 48, 1 + p % 48
                jw = wp - kw
                if 0 <= jw < 48:
                    wq[p, idx, :] = w4[:, 0, jj + 6 * s, jw]
    for c in range(NSP):
        for kw in range(3):
            for jh in range(12):
                idx = 18 + c * 36 + jh * 3 + kw   # j-major pass tiles
                for p in range(128):
                    flat = 128 * c + p
                    g, wp = flat // 48, 1 + flat % 48
                    jw = wp - kw
                    if 0 <= jw < 48:
                        wq[p, idx, :] = w4[:, 1 + g, jh, jw]
    wq = wq.reshape(128, NTILE * 128).astype(np.float16)

    bias = np.zeros((128, 8), dtype=np.float32)
    for c in range(NSP):
        for p in range(128):
            bias[p, c] = 3.5 - (128 * c + p) // 48
    # drain-time constant for removed w_pad 0 (kw=0) / 49 (kw=2) slots
    for g in range(8):
        bias[:, 4] += V0[g] * w4[:, 1 + g, :, 0].sum(axis=1)
        bias[:, 6] += V0[g] * w4[:, 1 + g, :, 47].sum(axis=1)
    return wq, bias


def _prep_x(x_slice):
    # x_slice: (B, CLOC, 48, 48) -> (xs [128, FREE], xp [128, NSP*FREE]) fp16
    # xs and xp[0] are bc-half-blocked: col = x*3200 + h*64 + b'
    # xp[1], xp[2] classic: col = h*128 + b
    plane = np.zeros((HP, HP, BC), dtype=np.float32)
    plane[1:49, 1:49, :] = np.ascontiguousarray(
        x_slice.transpose(3, 2, 0, 1)).reshape(48, 48, BC)
    flat = plane.reshape(HP, FREE)          # [w_pad, h*bc]
    sh6 = np.zeros_like(plane)              # h-shift by 6
    sh6[:, 0:44, :] = plane[:, 6:50, :]
    flat6 = sh6.reshape(HP, FREE)

    def blocked(a):  # [rows, h*bc] -> [rows, (x h b')]
        r = a.reshape(-1, HP, 2, 64)
        return np.ascontiguousarray(r.transpose(0, 2, 1, 3)).reshape(-1, FREE)

    xs = np.zeros((128, FREE), dtype=np.float16)
    xs[0:48] = blocked(flat[1:49])
    xs[48:96] = blocked(flat6[1:49])
    xp = np.empty((128, NSP * FREE), dtype=np.float16)
    for c in range(NSP):
        rows = [1 + (128 * c + p) % 48 for p in range(128)]
        xp[:, c * FREE:(c + 1) * FREE] = blocked(flat[rows])
    return xs, xp


def kernel(x, base_weight, spline_weight, spline_scaler):
    x = np.asarray(x, dtype=np.float32)
    wq, bias = _prep_weights(np.asarray(base_weight), np.asarray(spline_weight),
                             np.asarray(spline_scaler))
    nc = build_nc()
    in_maps = []
    for k in range(NCORES):
        xs, xp = _prep_x(x[:, k * CLOC:(k + 1) * CLOC])
        in_maps.append({"xs": xs, "xp": xp, "wq": wq, "bias": bias})
    res = run_bass_kernel_spmd(nc, in_maps, list(range(NCORES)), **RUN_KWARGS)
    global LAST_EXEC_NS
    LAST_EXEC_NS = res.exec_time_ns
    outs = [np.asarray(r["out"]) for r in res.results]

    full = np.empty((B, 2304, OUT_C), dtype=np.float32)
    for k in range(NCORES):
        dev = outs[k].astype(np.float32).reshape(3, 3, OUT_C, 4, B, CLOC)
        rows = dev.transpose(4, 5, 0, 1, 3, 2).reshape(B, 288, OUT_C)
        full[:, 288 * k:288 * (k + 1), :] = rows
    return full.reshape(B, 128, 2304).reshape(B, 128, 48, 48)


# revision 27
# speedup vs baseline: 1.0012x; 1.0012x over previous
"""ConvKAN Trainium2 kernel (v8: bc-half-blocked planes, all-halves waves).

Decomposition (validated vs reference):
  out[(b, cin, kh, kw, q), oc] =
      sum_{func, jh, jw} Wf[oc, func, jh*48+jw] * F_func(x_pad[b, cin, 12q+jh+kh, jw+kw])
  where F_0 = silu and F_{1+g}(v) = spline cubes 4*r1^3 - r2^3 with
  t = |2.5 v + 3.5 - g|, r2 = max(2-t, 0), r1 = max(1-t, 0)
  (weights carry the -1/6 normalization).

Sharding: input channels cin split 8 ways (8 per core); core k produces
output rows [288k, 288k+288) of (B, 2304, OUT_C).

Scheduling (measured ~107-109 us vs 153 us baseline):
- every function plane (and its x input) is stored bc-half-blocked
  [p, (x:2, h:50, b':64)], so the elementwise chains run on flat
  contiguous chunks (no strided-AP penalty) and every matmul wave runs
  as N=256 bc-halves with a contiguous rhs inner dim; a wave can start
  as soon as half a plane is ready.  Half-waves pace at the pure
  streaming rate (~109 ns) because kh-inner triples share lhsT.
- PSUM keeps the (q, b) layout; half waves write b-slices.  start=True
  only on the first matmul touching each bank, one stop on the last
  (per-element has_written semantics make partial-width accumulation
  sound).
- inputs stream on one HWDGE queue in consumption order: xs[h0] ->
  wq[silu] -> xs[h1] -> xp0 -> wq[p0,j<6] -> xp1[h0] -> wq[p0,j>=6] ->
  xp1[h1] -> wq[p1,j<6] -> xp2[h0] -> wq[p1,j>=6] -> xp2[h1] -> wq[p2].
- spline chains are emitted stage-major within each bc-half; square ops
  are split between ACT and DVE per knobs (front-loaded for pass-0 h0,
  the only remaining plane stall, ~2.5 us).
- zero-tile warmup matmuls keep the PE busy from engine init (HAM clock
  gate opens ~4 us before the first real wave), and zero-weight filler
  matmuls bridge the pass-0 plane wait so the clock never re-throttles.
- group 8 runs right after group 0's bank drains; outputs are fp16.
"""

from contextlib import ExitStack

import numpy as np

import concourse.bass as bass
import concourse.bacc as bacc
import concourse.tile as tile
from concourse import mybir
from concourse.alu_op_type import AluOpType
from concourse.bass_utils import run_bass_kernel_spmd

AF = mybir.ActivationFunctionType
DT = mybir.dt

B, C, H, W = 16, 64, 48, 48
OUT_C = 128
NCORES = 8
CLOC = C // NCORES          # 8 input channels per core
BC = B * CLOC               # 128 (b, c) pairs per core
HP = 50                     # padded height
FREE = HP * BC              # 6400
NSP = 3                     # spline passes
NTILE = 3 * 6 + NSP * 3 * 12  # 126 lhsT tiles: (silu kw jj) + (pass kw jh)
FCH = 4                     # chunks per activation pass (1600 cols each)
RUN_KWARGS = {}
LAST_EXEC_NS = None
N_WARMUP = 70               # HAM warm-up dummy matmuls (N=128, span the DMA wait)

# engine-assignment knobs, [pass][chunk]: square ops on ACT vs DVE
S2_ACT = ((False, False, True, True),
          (True, True, False, False),
          (True, True, False, False))
S1F_ACT = ((True, True, True, True),
           (True, True, True, True),
           (True, True, True, True))

V0 = (0.0, 0.0, -0.125, -2.875, -2.875, -0.125, 0.0, 0.0)  # slot value at x=0


def build_nc(fch: int = FCH) -> bass.Bass:
    nc = bacc.Bacc(None, target_bir_lowering=False, debug=True)
    xs = nc.declare_dram_parameter("xs", [128, FREE], DT.float16, isOutput=False)
    xp = nc.declare_dram_parameter("xp", [128, NSP * FREE], DT.float16,
                                   isOutput=False)
    wq = nc.declare_dram_parameter("wq", [128, NTILE * 128], DT.float16,
                                   isOutput=False)
    bias = nc.declare_dram_parameter("bias", [128, 8], DT.float32, isOutput=False)
    out = nc.declare_dram_parameter("out", [9, 128, 512], DT.float16, isOutput=True)

    fw = FREE // fch
    with ExitStack() as ctx:
        tc = ctx.enter_context(tile.TileContext(nc))
        wpool = ctx.enter_context(tc.tile_pool(name="w", bufs=1))
        fpool = ctx.enter_context(tc.tile_pool(name="f", bufs=3))
        psum_pool = ctx.enter_context(tc.tile_pool(name="ps", bufs=8, space="PSUM"))
        opool = ctx.enter_context(tc.tile_pool(name="o", bufs=4))

        bias_sb = wpool.tile([128, 8], DT.float32)
        nc.gpsimd.dma_start(bias_sb[:], bias[:])

        xs_sb = wpool.tile([128, FREE], DT.float16)
        wq_sb = wpool.tile([128, NTILE * 128], DT.float16)
        xp_sb = [wpool.tile([128, FREE], DT.float16, name=f"xp{c}", tag=f"xp{c}")
                 for c in range(NSP)]

        # input DMA stream in consumption order on the HWDGE queue
        def dma_chunks(dst, src_base, cs):
            for f in cs:
                nc.sync.dma_start(dst[:, f * fw:(f + 1) * fw],
                                  xp[:, src_base + f * fw:src_base + (f + 1) * fw]
                                  if src_base is not None else
                                  xs[:, f * fw:(f + 1) * fw])

        def dma_wq(t0, t1):
            nc.sync.dma_start(wq_sb[:, t0 * 128:t1 * 128], wq[:, t0 * 128:t1 * 128])

        dma_chunks(xs_sb, None, (0, 1))
        dma_wq(0, 18)                      # silu tiles
        dma_chunks(xs_sb, None, (2, 3))
        dma_chunks(xp_sb[0], 0, range(fch))
        dma_wq(18, 36)                     # p0 j0-5
        dma_chunks(xp_sb[1], FREE, (0, 1))
        dma_wq(36, 54)                     # p0 j6-11
        dma_chunks(xp_sb[1], FREE, (2, 3))
        dma_wq(54, 72)                     # p1 j0-5
        dma_chunks(xp_sb[2], 2 * FREE, (0, 1))
        dma_wq(72, 90)                     # p1 j6-11
        dma_chunks(xp_sb[2], 2 * FREE, (2, 3))
        dma_wq(90, 126)                    # p2

        ts_s = wpool.tile([128, FREE], DT.float16, name="tsS", tag="tsS")
        ts_t = [wpool.tile([128, FREE], DT.float16, name=f"ts{c}", tag=f"ts{c}")
                for c in range(NSP)]

        groups = [(kh, kw) for kh in range(3) for kw in range(3)]
        ps_tiles = {}
        for g in groups[:8]:
            ps_tiles[g] = psum_pool.tile([128, 512], DT.float32,
                                         name=f"ps_{g[0]}{g[1]}", tag="ps")
        zt = wpool.tile([128, 128], DT.float16, name="zt", tag="zt")
        nc.vector.memset(zt[:], 0.0)
        # HAM warm-up into group-7's bank (cleared by its first start=True mm);
        # zero-tile operands need no DMA, so the PE is busy (and the clock
        # gate open) from right after engine init until the real waves start
        warm = ps_tiles[groups[7]][:, 0:128]
        for _ in range(N_WARMUP):
            nc.tensor.matmul(warm, zt[:], zt[:], start=True, stop=False)

        # silu chain: one ACT op per chunk (chunks 0,1 = half 0)
        for f in range(fch):
            sl = slice(f * fw, (f + 1) * fw)
            nc.scalar.activation(ts_s[:, sl], xs_sb[:, sl], AF.Silu)

        # spline chains (flat chunks; stage-major within each bc-half so the
        # DVE pipeline is not serialized behind one chunk's whole chain)
        for c in range(NSP):
            bias_ap = bias_sb[:, c:c + 1]
            for half in (0, 1):
                chunks = (2 * half, 2 * half + 1)
                tl = {}
                for f in chunks:
                    sl = slice(f * fw, (f + 1) * fw)
                    t = fpool.tile([128, fw], DT.float16, name="t", tag="t")
                    nc.scalar.activation(t[:], xp_sb[c][:, sl], AF.Abs,
                                         bias=bias_ap, scale=2.5)
                    tl[f] = {"t": t}
                for f in chunks:
                    nr2 = fpool.tile([128, fw], DT.float16, name="nr2", tag="nr2")
                    nc.vector.tensor_scalar(nr2[:], tl[f]["t"][:], 2.0, 0.0,
                                            op0=AluOpType.subtract,
                                            op1=AluOpType.min)
                    nr1 = fpool.tile([128, fw], DT.float16, name="nr1", tag="nr1")
                    nc.vector.tensor_scalar(nr1[:], tl[f]["t"][:], 1.0, 0.0,
                                            op0=AluOpType.subtract,
                                            op1=AluOpType.min)
                    tl[f]["nr2"], tl[f]["nr1"] = nr2, nr1
                for f in chunks:
                    s2 = fpool.tile([128, fw], DT.float16, name="s2", tag="s2")
                    if S2_ACT[c][f]:
                        nc.scalar.activation(s2[:], tl[f]["nr2"][:], AF.Square)
                    else:
                        nc.vector.tensor_tensor(s2[:], tl[f]["nr2"][:],
                                                tl[f]["nr2"][:], op=AluOpType.mult)
                    tl[f]["s2"] = s2
                for f in chunks:
                    s1f = fpool.tile([128, fw], DT.float16, name="s1f", tag="s1f")
                    if S1F_ACT[c][f]:
                        nc.scalar.activation(s1f[:], tl[f]["nr1"][:], AF.Square,
                                             scale=2.0)
                    else:
                        nc.vector.scalar_tensor_tensor(
                            s1f[:], tl[f]["nr1"][:], 4.0, tl[f]["nr1"][:],
                            op0=AluOpType.mult, op1=AluOpType.mult)
                    tl[f]["s1f"] = s1f
                for f in chunks:
                    c2n = fpool.tile([128, fw], DT.float16, name="c2n", tag="c2n")
                    nc.vector.tensor_tensor(c2n[:], tl[f]["s2"][:],
                                            tl[f]["nr2"][:], op=AluOpType.mult)
                    tl[f]["c2n"] = c2n
                for f in chunks:
                    cn1 = fpool.tile([128, fw], DT.float16, name="cn1", tag="cn1")
                    nc.vector.tensor_tensor(cn1[:], tl[f]["s1f"][:],
                                            tl[f]["nr1"][:], op=AluOpType.mult)
                    tl[f]["cn1"] = cn1
                for f in chunks:
                    sl = slice(f * fw, (f + 1) * fw)
                    nc.vector.tensor_tensor(ts_t[c][:, sl], tl[f]["c2n"][:],
                                            tl[f]["cn1"][:], op=AluOpType.subtract)

        def emit_mm(g, seq, x, start=False, stop=False):
            kh, kw = g
            kind, c, j = seq
            if kind == "S":
                idx = kw * 6 + j
                src = ts_s
            else:
                idx = 18 + c * 36 + j * 3 + kw     # j-major pass tiles
                src = ts_t[c]
            lhsT = wq_sb[:, idx * 128:(idx + 1) * 128]
            h0 = kh + j
            ps = ps_tiles[g][:].rearrange("p (q b) -> p q b", b=BC)
            rhs = src[:].rearrange("p (x h b) -> p x h b", x=2, b=64)[
                :, x, h0:h0 + 37:12, :]
            out_ap = ps[:, :, 64 * x:64 * (x + 1)]
            nc.tensor.matmul(out_ap, lhsT, rhs, start=start, stop=stop)

        def drain(g):
            ob = opool.tile([128, 512], DT.float16)
            # adds the constant contribution of the removed w_pad 0/49 slots;
            # on DVE (idle, empty queue at the tail) so the final drain
            # dispatches right after its group's stop matmul.  Split in
            # halves so the first out-DMA overlaps the second half's drain.
            for h0, h1 in ((0, 256), (256, 512)):
                nc.vector.tensor_scalar(ob[:, h0:h1], ps_tiles[g][:, h0:h1],
                                        bias_sb[:, 4 + g[1]:5 + g[1]], None,
                                        op0=AluOpType.add)
                nc.sync.dma_start(out[g[0] * 3 + g[1]][:, h0:h1], ob[:, h0:h1])

        wave = groups[:8]
        silu_seqs = [("S", 0, j) for j in range(6)]
        pass_seqs = [[("P", c, j) for j in range(12)] for c in range(NSP)]

        def kworder(include_g8=False):
            gs = groups[:9] if include_g8 else groups[:8]
            return sorted(gs, key=lambda g: (g[1], g[0]))  # kw major, kh inner

        # per bc-half waves; kh-inner triples share lhsT
        for x in (0, 1):
            for j, s in enumerate(silu_seqs):
                for g in kworder():
                    emit_mm(g, s, x, start=(x == 0 and j == 0))
        # zero-weight fillers keep the PE (and its HAM clock state) busy
        # through the pass-0 plane wait; they add 0 to a live bank
        for _ in range(24):
            nc.tensor.matmul(ps_tiles[groups[0]][:, 0:256], zt[:],
                             ts_s[:, 0:256], start=False, stop=False)
        for c in (0, 1):
            for x in (0, 1):
                for s in pass_seqs[c]:
                    for g in kworder():
                        emit_mm(g, s, x)
        # pass 2: group 0 first, drain it, then g8's full run, then the rest
        for x in (0, 1):
            for i, s in enumerate(pass_seqs[2]):
                emit_mm(wave[0], s, x, stop=(x == 1 and i == 11))
        drain(wave[0])
        g8 = groups[8]
        ps_tiles[g8] = psum_pool.tile([128, 512], DT.float32, name="ps_22",
                                      tag="ps")
        g8_seqs = [(s, x) for s in silu_seqs + pass_seqs[0] + pass_seqs[1]
                   + pass_seqs[2] for x in (0, 1)]
        for i, (s, x) in enumerate(g8_seqs):
            emit_mm(g8, s, x, start=(i == 0), stop=(i == len(g8_seqs) - 1))
        drain(g8)
        for g in [g for g in kworder() if g != wave[0]]:
            for x in (0, 1):
                for i, s in enumerate(pass_seqs[2]):
                    emit_mm(g, s, x, stop=(x == 1 and i == 11))
            drain(g)
    nc.compile()
    return nc


def _prep_weights(base_weight, spline_weight, spline_scaler):
    # Wf[oc, func, jj]: func 0 = silu weights, 1+g = scaled spline / -6
    wf = np.empty((OUT_C, 9, 576), dtype=np.float64)
    wf[:, 0, :] = base_weight
    wf[:, 1:, :] = np.moveaxis(
        spline_weight.astype(np.float64)
        * spline_scaler.astype(np.float64)[..., None] / -6.0, -1, 1)
    w4 = wf.reshape(OUT_C, 9, 12, 48)
    wq = np.zeros((128, NTILE, OUT_C), dtype=np.float64)
    for kw in range(3):
        for jj in range(6):  # silu tiles
            idx = kw * 6 + jj
            for p in range(96):
                s, wp = p // 48, 1 + p % 48
                jw = wp - kw
                if 0 <= jw < 48:
                    wq[p, idx, :] = w4[:, 0, jj + 6 * s, jw]
    for c in range(NSP):
        for kw in range(3):
            for jh in range(12):
                idx = 18 + c * 36 + jh * 3 + kw   # j-major pass tiles
                for p in range(128):
                    flat = 128 * c + p
                    g, wp = flat // 48, 1 + flat % 48
                    jw = wp - kw
                    if 0 <= jw < 48:
                        wq[p, idx, :] = w4[:, 1 + g, jh, jw]
    wq = wq.reshape(128, NTILE * 128).astype(np.float16)

    bias = np.zeros((128, 8), dtype=np.float32)
    for c in range(NSP):
        for p in range(128):
            bias[p, c] = 3.5 - (128 * c + p) // 48
    # drain-time constant for removed w_pad 0 (kw=0) / 49 (kw=2) slots
    for g in range(8):
        bias[:, 4] += V0[g] * w4[:, 1 + g, :, 0].sum(axis=1)
        bias[:, 6] += V0[g] * w4[:, 1 + g, :, 47].sum(axis=1)
    return wq, bias


def _prep_x(x_slice):
    # x_slice: (B, CLOC, 48, 48) -> (xs [128, FREE], xp [128, NSP*FREE]) fp16
    # xs and xp[0] are bc-half-blocked: col = x*3200 + h*64 + b'
    # xp[1], xp[2] classic: col = h*128 + b
    plane = np.zeros((HP, HP, BC), dtype=np.float32)
    plane[1:49, 1:49, :] = np.ascontiguousarray(
        x_slice.transpose(3, 2, 0, 1)).reshape(48, 48, BC)
    flat = plane.reshape(HP, FREE)          # [w_pad, h*bc]
    sh6 = np.zeros_like(plane)              # h-shift by 6
    sh6[:, 0:44, :] = plane[:, 6:50, :]
    flat6 = sh6.reshape(HP, FREE)

    def blocked(a):  # [rows, h*bc] -> [rows, (x h b')]
        r = a.reshape(-1, HP, 2, 64)
        return np.ascontiguousarray(r.transpose(0, 2, 1, 3)).reshape(-1, FREE)

    xs = np.zeros((128, FREE), dtype=np.float16)
    xs[0:48] = blocked(flat[1:49])
    xs[48:96] = blocked(flat6[1:49])
    xp = np.empty((128, NSP * FREE), dtype=np.float16)
    for c in range(NSP):
        rows = [1 + (128 * c + p) % 48 for p in range(128)]
        xp[:, c * FREE:(c + 1) * FREE] = blocked(flat[rows])
    return xs, xp


def kernel(x, base_weight, spline_weight, spline_scaler):
    x = np.asarray(x, dtype=np.float32)
    wq, bias = _prep_weights(np.asarray(base_weight), np.asarray(spline_weight),
                             np.asarray(spline_scaler))
    nc = build_nc()
    in_maps = []
    for k in range(NCORES):
        xs, xp = _prep_x(x[:, k * CLOC:(k + 1) * CLOC])
        in_maps.append({"xs": xs, "xp": xp, "wq": wq, "bias": bias})
    res = run_bass_kernel_spmd(nc, in_maps, list(range(NCORES)), **RUN_KWARGS)
    global LAST_EXEC_NS
    LAST_EXEC_NS = res.exec_time_ns
    outs = [np.asarray(r["out"]) for r in res.results]

    full = np.empty((B, 2304, OUT_C), dtype=np.float32)
    for k in range(NCORES):
        dev = outs[k].astype(np.float32).reshape(3, 3, OUT_C, 4, B, CLOC)
        rows = dev.transpose(4, 5, 0, 1, 3, 2).reshape(B, 288, OUT_C)
        full[:, 288 * k:288 * (k + 1), :] = rows
    return full.reshape(B, 128, 2304).reshape(B, 128, 48, 48)


# revision 30
# speedup vs baseline: 1.0152x; 1.0140x over previous
"""ConvKAN Trainium2 kernel (v8: bc-half-blocked planes, all-halves waves).

Decomposition (validated vs reference):
  out[(b, cin, kh, kw, q), oc] =
      sum_{func, jh, jw} Wf[oc, func, jh*48+jw] * F_func(x_pad[b, cin, 12q+jh+kh, jw+kw])
  where F_0 = silu and F_{1+g}(v) = spline cubes 4*r1^3 - r2^3 with
  t = |2.5 v + 3.5 - g|, r2 = max(2-t, 0), r1 = max(1-t, 0)
  (weights carry the -1/6 normalization).

Sharding: input channels cin split 8 ways (8 per core); core k produces
output rows [288k, 288k+288) of (B, 2304, OUT_C).

Scheduling (measured ~107-109 us vs 153 us baseline):
- every function plane (and its x input) is stored bc-half-blocked
  [p, (x:2, h:50, b':64)], so the elementwise chains run on flat
  contiguous chunks (no strided-AP penalty) and every matmul wave runs
  as N=256 bc-halves with a contiguous rhs inner dim; a wave can start
  as soon as half a plane is ready.  Half-waves pace at the pure
  streaming rate (~109 ns) because kh-inner triples share lhsT.
- PSUM keeps the (q, b) layout; half waves write b-slices.  start=True
  only on the first matmul touching each bank, one stop on the last
  (per-element has_written semantics make partial-width accumulation
  sound).
- inputs stream on one HWDGE queue in consumption order: xs[h0] ->
  wq[silu] -> xs[h1] -> xp0 -> wq[p0,j<6] -> xp1[h0] -> wq[p0,j>=6] ->
  xp1[h1] -> wq[p1,j<6] -> xp2[h0] -> wq[p1,j>=6] -> xp2[h1] -> wq[p2].
- spline chains are emitted stage-major within each bc-half; square ops
  are split between ACT and DVE per knobs (front-loaded for pass-0 h0,
  the only remaining plane stall, ~2.5 us).
- zero-tile warmup matmuls keep the PE busy from engine init (HAM clock
  gate opens ~4 us before the first real wave), and zero-weight filler
  matmuls bridge the pass-0 plane wait so the clock never re-throttles.
- group 8 runs right after group 0's bank drains; outputs are fp16.
"""

from contextlib import ExitStack

import numpy as np

import concourse.bass as bass
import concourse.bacc as bacc
import concourse.tile as tile
from concourse import mybir
from concourse.alu_op_type import AluOpType
from concourse.bass_utils import run_bass_kernel_spmd

AF = mybir.ActivationFunctionType
DT = mybir.dt

B, C, H, W = 16, 64, 48, 48
OUT_C = 128
NCORES = 8
CLOC = C // NCORES          # 8 input channels per core
BC = B * CLOC               # 128 (b, c) pairs per core
HP = 50                     # padded height
FREE = HP * BC              # 6400
NSP = 3                     # spline passes
NTILE = 3 * 6 + NSP * 3 * 12  # 126 lhsT tiles: (silu kw jj) + (pass kw jh)
FCH = 4                     # chunks per activation pass (1600 cols each)
RUN_KWARGS = {}
LAST_EXEC_NS = None
N_WARMUP = 70               # HAM warm-up dummy matmuls (N=128, span the DMA wait)

# engine-assignment knobs, [pass][chunk]: square ops on ACT vs DVE
S2_ACT = ((False, False, True, True),
          (True, True, False, False),
          (True, True, False, False))
S1F_ACT = ((True, True, True, True),
           (True, True, True, True),
           (True, True, True, True))

V0 = (0.0, 0.0, -0.125, -2.875, -2.875, -0.125, 0.0, 0.0)  # slot value at x=0


def build_nc(fch: int = FCH) -> bass.Bass:
    nc = bacc.Bacc(None, target_bir_lowering=False, debug=True)
    xs = nc.declare_dram_parameter("xs", [128, FREE], DT.float16, isOutput=False)
    xp = nc.declare_dram_parameter("xp", [128, NSP * FREE], DT.float16,
                                   isOutput=False)
    wq = nc.declare_dram_parameter("wq", [128, NTILE * 128], DT.float16,
                                   isOutput=False)
    bias = nc.declare_dram_parameter("bias", [128, 8], DT.float32, isOutput=False)
    out = nc.declare_dram_parameter("out", [9, 128, 512], DT.float16, isOutput=True)

    fw = FREE // fch
    with ExitStack() as ctx:
        tc = ctx.enter_context(tile.TileContext(nc))
        wpool = ctx.enter_context(tc.tile_pool(name="w", bufs=1))
        fpool = ctx.enter_context(tc.tile_pool(name="f", bufs=3))
        psum_pool = ctx.enter_context(tc.tile_pool(name="ps", bufs=8, space="PSUM"))
        opool = ctx.enter_context(tc.tile_pool(name="o", bufs=4))

        bias_sb = wpool.tile([128, 8], DT.float32)
        nc.gpsimd.dma_start(bias_sb[:], bias[:])

        xs_sb = wpool.tile([128, FREE], DT.float16)
        wq_sb = wpool.tile([128, NTILE * 128], DT.float16)
        xp_sb = [wpool.tile([128, FREE], DT.float16, name=f"xp{c}", tag=f"xp{c}")
                 for c in range(NSP)]

        # input DMA stream in consumption order on the HWDGE queue
        def dma_chunks(dst, src_base, cs):
            for f in cs:
                nc.sync.dma_start(dst[:, f * fw:(f + 1) * fw],
                                  xp[:, src_base + f * fw:src_base + (f + 1) * fw]
                                  if src_base is not None else
                                  xs[:, f * fw:(f + 1) * fw])

        def dma_wq(t0, t1):
            nc.sync.dma_start(wq_sb[:, t0 * 128:t1 * 128], wq[:, t0 * 128:t1 * 128])

        dma_chunks(xs_sb, None, (0, 1))
        dma_wq(0, 18)                      # silu tiles
        dma_chunks(xp_sb[0], 0, (0,))      # pass-0 chunk 0 early (see abs0)
        dma_chunks(xs_sb, None, (2, 3))
        dma_chunks(xp_sb[0], 0, (1, 2, 3))
        dma_wq(18, 36)                     # p0 j0-5
        dma_chunks(xp_sb[1], FREE, (0, 1))
        dma_wq(36, 54)                     # p0 j6-11
        dma_chunks(xp_sb[1], FREE, (2, 3))
        dma_wq(54, 72)                     # p1 j0-5
        dma_chunks(xp_sb[2], 2 * FREE, (0, 1))
        dma_wq(72, 90)                     # p1 j6-11
        dma_chunks(xp_sb[2], 2 * FREE, (2, 3))
        dma_wq(90, 126)                    # p2

        ts_s = wpool.tile([128, FREE], DT.float16, name="tsS", tag="tsS")
        ts_t = [wpool.tile([128, FREE], DT.float16, name=f"ts{c}", tag=f"ts{c}")
                for c in range(NSP)]

        groups = [(kh, kw) for kh in range(3) for kw in range(3)]
        ps_tiles = {}
        for g in groups[:8]:
            ps_tiles[g] = psum_pool.tile([128, 512], DT.float32,
                                         name=f"ps_{g[0]}{g[1]}", tag="ps")
        zt = wpool.tile([128, 128], DT.float16, name="zt", tag="zt")
        nc.vector.memset(zt[:], 0.0)
        # HAM warm-up into group-7's bank (cleared by its first start=True mm);
        # zero-tile operands need no DMA, so the PE is busy (and the clock
        # gate open) from right after engine init until the real waves start
        warm = ps_tiles[groups[7]][:, 0:128]
        for _ in range(N_WARMUP):
            nc.tensor.matmul(warm, zt[:], zt[:], start=True, stop=False)

        # silu chain: one ACT op per chunk (chunks 0,1 = half 0).  Pass-0's
        # first abs is interleaved after the h0 silu ops so the DVE chain for
        # the pass-0 h0 plane starts ~3.5us earlier (ACT is otherwise the
        # serial gate); the silu h1 plane still lands before its wave needs it.
        pre_abs = {}
        for f in range(fch):
            sl = slice(f * fw, (f + 1) * fw)
            nc.scalar.activation(ts_s[:, sl], xs_sb[:, sl], AF.Silu)
            if f == 1:
                t0 = fpool.tile([128, fw], DT.float16, name="t", tag="t")
                nc.scalar.activation(t0[:], xp_sb[0][:, 0:fw], AF.Abs,
                                     bias=bias_sb[:, 0:1], scale=2.5)
                pre_abs[0] = t0

        # spline chains (flat chunks; stage-major within each bc-half so the
        # DVE pipeline is not serialized behind one chunk's whole chain)
        for c in range(NSP):
            bias_ap = bias_sb[:, c:c + 1]
            for half in (0, 1):
                chunks = (2 * half, 2 * half + 1)
                tl = {}
                for f in chunks:
                    if c == 0 and f in pre_abs:
                        tl[f] = {"t": pre_abs[f]}
                        continue
                    sl = slice(f * fw, (f + 1) * fw)
                    t = fpool.tile([128, fw], DT.float16, name="t", tag="t")
                    nc.scalar.activation(t[:], xp_sb[c][:, sl], AF.Abs,
                                         bias=bias_ap, scale=2.5)
                    tl[f] = {"t": t}
                for f in chunks:
                    nr2 = fpool.tile([128, fw], DT.float16, name="nr2", tag="nr2")
                    nc.vector.tensor_scalar(nr2[:], tl[f]["t"][:], 2.0, 0.0,
                                            op0=AluOpType.subtract,
                                            op1=AluOpType.min)
                    nr1 = fpool.tile([128, fw], DT.float16, name="nr1", tag="nr1")
                    nc.vector.tensor_scalar(nr1[:], tl[f]["t"][:], 1.0, 0.0,
                                            op0=AluOpType.subtract,
                                            op1=AluOpType.min)
                    tl[f]["nr2"], tl[f]["nr1"] = nr2, nr1
                for f in chunks:
                    s2 = fpool.tile([128, fw], DT.float16, name="s2", tag="s2")
                    if S2_ACT[c][f]:
                        nc.scalar.activation(s2[:], tl[f]["nr2"][:], AF.Square)
                    else:
                        nc.vector.tensor_tensor(s2[:], tl[f]["nr2"][:],
                                                tl[f]["nr2"][:], op=AluOpType.mult)
                    tl[f]["s2"] = s2
                for f in chunks:
                    s1f = fpool.tile([128, fw], DT.float16, name="s1f", tag="s1f")
                    if S1F_ACT[c][f]:
                        nc.scalar.activation(s1f[:], tl[f]["nr1"][:], AF.Square,
                                             scale=2.0)
                    else:
                        nc.vector.scalar_tensor_tensor(
                            s1f[:], tl[f]["nr1"][:], 4.0, tl[f]["nr1"][:],
                            op0=AluOpType.mult, op1=AluOpType.mult)
                    tl[f]["s1f"] = s1f
                for f in chunks:
                    c2n = fpool.tile([128, fw], DT.float16, name="c2n", tag="c2n")
                    nc.vector.tensor_tensor(c2n[:], tl[f]["s2"][:],
                                            tl[f]["nr2"][:], op=AluOpType.mult)
                    tl[f]["c2n"] = c2n
                for f in chunks:
                    cn1 = fpool.tile([128, fw], DT.float16, name="cn1", tag="cn1")
                    nc.vector.tensor_tensor(cn1[:], tl[f]["s1f"][:],
                                            tl[f]["nr1"][:], op=AluOpType.mult)
                    tl[f]["cn1"] = cn1
                for f in chunks:
                    sl = slice(f * fw, (f + 1) * fw)
                    nc.vector.tensor_tensor(ts_t[c][:, sl], tl[f]["c2n"][:],
                                            tl[f]["cn1"][:], op=AluOpType.subtract)

        def emit_mm(g, seq, x, start=False, stop=False):
            kh, kw = g
            kind, c, j = seq
            if kind == "S":
                idx = kw * 6 + j
                src = ts_s
            else:
                idx = 18 + c * 36 + j * 3 + kw     # j-major pass tiles
                src = ts_t[c]
            lhsT = wq_sb[:, idx * 128:(idx + 1) * 128]
            h0 = kh + j
            ps = ps_tiles[g][:].rearrange("p (q b) -> p q b", b=BC)
            rhs = src[:].rearrange("p (x h b) -> p x h b", x=2, b=64)[
                :, x, h0:h0 + 37:12, :]
            out_ap = ps[:, :, 64 * x:64 * (x + 1)]
            nc.tensor.matmul(out_ap, lhsT, rhs, start=start, stop=stop)

        def drain(g):
            ob = opool.tile([128, 512], DT.float16)
            # adds the constant contribution of the removed w_pad 0/49 slots;
            # on DVE (idle, empty queue at the tail) so the final drain
            # dispatches right after its group's stop matmul.  Split in
            # halves so the first out-DMA overlaps the second half's drain.
            for h0, h1 in ((0, 256), (256, 512)):
                nc.vector.tensor_scalar(ob[:, h0:h1], ps_tiles[g][:, h0:h1],
                                        bias_sb[:, 4 + g[1]:5 + g[1]], None,
                                        op0=AluOpType.add)
                nc.sync.dma_start(out[g[0] * 3 + g[1]][:, h0:h1], ob[:, h0:h1])

        wave = groups[:8]
        silu_seqs = [("S", 0, j) for j in range(6)]
        pass_seqs = [[("P", c, j) for j in range(12)] for c in range(NSP)]

        def kworder(include_g8=False):
            gs = groups[:9] if include_g8 else groups[:8]
            return sorted(gs, key=lambda g: (g[1], g[0]))  # kw major, kh inner

        # per bc-half waves; kh-inner triples share lhsT
        for x in (0, 1):
            for j, s in enumerate(silu_seqs):
                for g in kworder():
                    emit_mm(g, s, x, start=(x == 0 and j == 0))
        # zero-weight fillers keep the PE (and its HAM clock state) busy
        # through the pass-0 plane wait; they add 0 to a live bank
        for _ in range(24):
            nc.tensor.matmul(ps_tiles[groups[0]][:, 0:256], zt[:],
                             ts_s[:, 0:256], start=False, stop=False)
        for c in (0, 1):
            for x in (0, 1):
                for s in pass_seqs[c]:
                    for g in kworder():
                        emit_mm(g, s, x)
        # pass 2: group 0 first, drain it, then g8's full run, then the rest
        for x in (0, 1):
            for i, s in enumerate(pass_seqs[2]):
                emit_mm(wave[0], s, x, stop=(x == 1 and i == 11))
        drain(wave[0])
        g8 = groups[8]
        ps_tiles[g8] = psum_pool.tile([128, 512], DT.float32, name="ps_22",
                                      tag="ps")
        g8_seqs = [(s, x) for s in silu_seqs + pass_seqs[0] + pass_seqs[1]
                   + pass_seqs[2] for x in (0, 1)]
        for i, (s, x) in enumerate(g8_seqs):
            emit_mm(g8, s, x, start=(i == 0), stop=(i == len(g8_seqs) - 1))
        drain(g8)
        for g in [g for g in kworder() if g != wave[0]]:
            for x in (0, 1):
                for i, s in enumerate(pass_seqs[2]):
                    emit_mm(g, s, x, stop=(x == 1 and i == 11))
            drain(g)
    nc.compile()
    return nc


def _prep_weights(base_weight, spline_weight, spline_scaler):
    # Wf[oc, func, jj]: func 0 = silu weights, 1+g = scaled spline / -6
    wf = np.empty((OUT_C, 9, 576), dtype=np.float64)
    wf[:, 0, :] = base_weight
    wf[:, 1:, :] = np.moveaxis(
        spline_weight.astype(np.float64)
        * spline_scaler.astype(np.float64)[..., None] / -6.0, -1, 1)
    w4 = wf.reshape(OUT_C, 9, 12, 48)
    wq = np.zeros((128, NTILE, OUT_C), dtype=np.float64)
    for kw in range(3):
        for jj in range(6):  # silu tiles
            idx = kw * 6 + jj
            for p in range(96):
                s, wp = p // 48, 1 + p % 48
                jw = wp - kw
                if 0 <= jw < 48:
                    wq[p, idx, :] = w4[:, 0, jj + 6 * s, jw]
    for c in range(NSP):
        for kw in range(3):
            for jh in range(12):
                idx = 18 + c * 36 + jh * 3 + kw   # j-major pass tiles
                for p in range(128):
                    flat = 128 * c + p
                    g, wp = flat // 48, 1 + flat % 48
                    jw = wp - kw
                    if 0 <= jw < 48:
                        wq[p, idx, :] = w4[:, 1 + g, jh, jw]
    wq = wq.reshape(128, NTILE * 128).astype(np.float16)

    bias = np.zeros((128, 8), dtype=np.float32)
    for c in range(NSP):
        for p in range(128):
            bias[p, c] = 3.5 - (128 * c + p) // 48
    # drain-time constant for removed w_pad 0 (kw=0) / 49 (kw=2) slots
    for g in range(8):
        bias[:, 4] += V0[g] * w4[:, 1 + g, :, 0].sum(axis=1)
        bias[:, 6] += V0[g] * w4[:, 1 + g, :, 47].sum(axis=1)
    return wq, bias


def _prep_x(x_slice):
    # x_slice: (B, CLOC, 48, 48) -> (xs [128, FREE], xp [128, NSP*FREE]) fp16
    # xs and xp[0] are bc-half-blocked: col = x*3200 + h*64 + b'
    # xp[1], xp[2] classic: col = h*128 + b
    plane = np.zeros((HP, HP, BC), dtype=np.float32)
    plane[1:49, 1:49, :] = np.ascontiguousarray(
        x_slice.transpose(3, 2, 0, 1)).reshape(48, 48, BC)
    flat = plane.reshape(HP, FREE)          # [w_pad, h*bc]
    sh6 = np.zeros_like(plane)              # h-shift by 6
    sh6[:, 0:44, :] = plane[:, 6:50, :]
    flat6 = sh6.reshape(HP, FREE)

    def blocked(a):  # [rows, h*bc] -> [rows, (x h b')]
        r = a.reshape(-1, HP, 2, 64)
        return np.ascontiguousarray(r.transpose(0, 2, 1, 3)).reshape(-1, FREE)

    xs = np.zeros((128, FREE), dtype=np.float16)
    xs[0:48] = blocked(flat[1:49])
    xs[48:96] = blocked(flat6[1:49])
    xp = np.empty((128, NSP * FREE), dtype=np.float16)
    for c in range(NSP):
        rows = [1 + (128 * c + p) % 48 for p in range(128)]
        xp[:, c * FREE:(c + 1) * FREE] = blocked(flat[rows])
    return xs, xp


def kernel(x, base_weight, spline_weight, spline_scaler):
    x = np.asarray(x, dtype=np.float32)
    wq, bias = _prep_weights(np.asarray(base_weight), np.asarray(spline_weight),
                             np.asarray(spline_scaler))
    nc = build_nc()
    in_maps = []
    for k in range(NCORES):
        xs, xp = _prep_x(x[:, k * CLOC:(k + 1) * CLOC])
        in_maps.append({"xs": xs, "xp": xp, "wq": wq, "bias": bias})
    res = run_bass_kernel_spmd(nc, in_maps, list(range(NCORES)), **RUN_KWARGS)
    global LAST_EXEC_NS
    LAST_EXEC_NS = res.exec_time_ns
    outs = [np.asarray(r["out"]) for r in res.results]

    full = np.empty((B, 2304, OUT_C), dtype=np.float32)
    for k in range(NCORES):
        dev = outs[k].astype(np.float32).reshape(3, 3, OUT_C, 4, B, CLOC)
        rows = dev.transpose(4, 5, 0, 1, 3, 2).reshape(B, 288, OUT_C)
        full[:, 288 * k:288 * (k + 1), :] = rows
    return full.reshape(B, 128, 2304).reshape(B, 128, 48, 48)


# revision 32
# speedup vs baseline: 1.0152x; 1.0001x over previous
"""ConvKAN Trainium2 kernel (v8: bc-half-blocked planes, all-halves waves).

Decomposition (validated vs reference):
  out[(b, cin, kh, kw, q), oc] =
      sum_{func, jh, jw} Wf[oc, func, jh*48+jw] * F_func(x_pad[b, cin, 12q+jh+kh, jw+kw])
  where F_0 = silu and F_{1+g}(v) = spline cubes 4*r1^3 - r2^3 with
  t = |2.5 v + 3.5 - g|, r2 = max(2-t, 0), r1 = max(1-t, 0)
  (weights carry the -1/6 normalization).

Sharding: input channels cin split 8 ways (8 per core); core k produces
output rows [288k, 288k+288) of (B, 2304, OUT_C).

Scheduling (measured ~107-109 us vs 153 us baseline):
- every function plane (and its x input) is stored bc-half-blocked
  [p, (x:2, h:50, b':64)], so the elementwise chains run on flat
  contiguous chunks (no strided-AP penalty) and every matmul wave runs
  as N=256 bc-halves with a contiguous rhs inner dim; a wave can start
  as soon as half a plane is ready.  Half-waves pace at the pure
  streaming rate (~109 ns) because kh-inner triples share lhsT.
- PSUM keeps the (q, b) layout; half waves write b-slices.  start=True
  only on the first matmul touching each bank, one stop on the last
  (per-element has_written semantics make partial-width accumulation
  sound).
- inputs stream on one HWDGE queue in consumption order: xs[h0] ->
  wq[silu] -> xs[h1] -> xp0 -> wq[p0,j<6] -> xp1[h0] -> wq[p0,j>=6] ->
  xp1[h1] -> wq[p1,j<6] -> xp2[h0] -> wq[p1,j>=6] -> xp2[h1] -> wq[p2].
- spline chains are emitted stage-major within each bc-half; square ops
  are split between ACT and DVE per knobs (front-loaded for pass-0 h0,
  the only remaining plane stall, ~2.5 us).
- zero-tile warmup matmuls keep the PE busy from engine init (HAM clock
  gate opens ~4 us before the first real wave), and zero-weight filler
  matmuls bridge the pass-0 plane wait so the clock never re-throttles.
- group 8 runs right after group 0's bank drains; outputs are fp16.
"""

from contextlib import ExitStack

import numpy as np

import concourse.bass as bass
import concourse.bacc as bacc
import concourse.tile as tile
from concourse import mybir
from concourse.alu_op_type import AluOpType
from concourse.bass_utils import run_bass_kernel_spmd

AF = mybir.ActivationFunctionType
DT = mybir.dt

B, C, H, W = 16, 64, 48, 48
OUT_C = 128
NCORES = 8
CLOC = C // NCORES          # 8 input channels per core
BC = B * CLOC               # 128 (b, c) pairs per core
HP = 50                     # padded height
FREE = HP * BC              # 6400
NSP = 3                     # spline passes
NTILE = 3 * 6 + NSP * 3 * 12  # 126 lhsT tiles: (silu kw jj) + (pass kw jh)
FCH = 4                     # chunks per activation pass (1600 cols each)
RUN_KWARGS = {}
LAST_EXEC_NS = None
N_WARMUP = 70               # HAM warm-up dummy matmuls (N=128, span the DMA wait)

# engine-assignment knobs, [pass][chunk]: square ops on ACT vs DVE
S2_ACT = ((False, False, True, True),
          (True, True, False, False),
          (True, True, False, False))
S1F_ACT = ((True, True, True, True),
           (True, True, True, True),
           (True, True, True, True))

V0 = (0.0, 0.0, -0.125, -2.875, -2.875, -0.125, 0.0, 0.0)  # slot value at x=0


def build_nc(fch: int = FCH) -> bass.Bass:
    nc = bacc.Bacc(None, target_bir_lowering=False, debug=True)
    xs = nc.declare_dram_parameter("xs", [128, FREE], DT.float16, isOutput=False)
    xp = nc.declare_dram_parameter("xp", [128, NSP * FREE], DT.float16,
                                   isOutput=False)
    wq = nc.declare_dram_parameter("wq", [128, NTILE * 128], DT.float16,
                                   isOutput=False)
    bias = nc.declare_dram_parameter("bias", [128, 8], DT.float32, isOutput=False)
    out = nc.declare_dram_parameter("out", [9, 128, 512], DT.float16, isOutput=True)

    fw = FREE // fch
    with ExitStack() as ctx:
        tc = ctx.enter_context(tile.TileContext(nc))
        wpool = ctx.enter_context(tc.tile_pool(name="w", bufs=1))
        fpool = ctx.enter_context(tc.tile_pool(name="f", bufs=3))
        psum_pool = ctx.enter_context(tc.tile_pool(name="ps", bufs=8, space="PSUM"))
        opool = ctx.enter_context(tc.tile_pool(name="o", bufs=4))

        bias_sb = wpool.tile([128, 8], DT.float32)
        nc.gpsimd.dma_start(bias_sb[:], bias[:])

        xs_sb = wpool.tile([128, FREE], DT.float16)
        wq_sb = wpool.tile([128, NTILE * 128], DT.float16)
        xp_sb = [wpool.tile([128, FREE], DT.float16, name=f"xp{c}", tag=f"xp{c}")
                 for c in range(NSP)]

        # input DMA stream in consumption order on the HWDGE queue
        def dma_chunks(dst, src_base, cs):
            for f in cs:
                nc.sync.dma_start(dst[:, f * fw:(f + 1) * fw],
                                  xp[:, src_base + f * fw:src_base + (f + 1) * fw]
                                  if src_base is not None else
                                  xs[:, f * fw:(f + 1) * fw])

        def dma_wq(t0, t1):
            nc.sync.dma_start(wq_sb[:, t0 * 128:t1 * 128], wq[:, t0 * 128:t1 * 128])

        dma_chunks(xs_sb, None, (0, 1))
        dma_wq(0, 18)                      # silu tiles
        dma_chunks(xs_sb, None, (2,))
        dma_chunks(xp_sb[0], 0, (0,))      # pass-0 chunk 0 early (see abs0)
        dma_chunks(xs_sb, None, (3,))
        dma_chunks(xp_sb[0], 0, (1, 2, 3))
        dma_wq(18, 36)                     # p0 j0-5
        dma_chunks(xp_sb[1], FREE, (0, 1))
        dma_wq(36, 54)                     # p0 j6-11
        dma_chunks(xp_sb[1], FREE, (2, 3))
        dma_wq(54, 72)                     # p1 j0-5
        dma_chunks(xp_sb[2], 2 * FREE, (0, 1))
        dma_wq(72, 90)                     # p1 j6-11
        dma_chunks(xp_sb[2], 2 * FREE, (2, 3))
        dma_wq(90, 126)                    # p2

        ts_s = wpool.tile([128, FREE], DT.float16, name="tsS", tag="tsS")
        ts_t = [wpool.tile([128, FREE], DT.float16, name=f"ts{c}", tag=f"ts{c}")
                for c in range(NSP)]

        groups = [(kh, kw) for kh in range(3) for kw in range(3)]
        ps_tiles = {}
        for g in groups[:8]:
            ps_tiles[g] = psum_pool.tile([128, 512], DT.float32,
                                         name=f"ps_{g[0]}{g[1]}", tag="ps")
        zt = wpool.tile([128, 128], DT.float16, name="zt", tag="zt")
        nc.vector.memset(zt[:], 0.0)
        # HAM warm-up into group-7's bank (cleared by its first start=True mm);
        # zero-tile operands need no DMA, so the PE is busy (and the clock
        # gate open) from right after engine init until the real waves start
        warm = ps_tiles[groups[7]][:, 0:128]
        for _ in range(N_WARMUP):
            nc.tensor.matmul(warm, zt[:], zt[:], start=True, stop=False)

        # silu chain: one ACT op per chunk (chunks 0,1 = half 0).  Pass-0's
        # first abs is interleaved after the h0 silu ops so the DVE chain for
        # the pass-0 h0 plane starts ~3.5us earlier (ACT is otherwise the
        # serial gate); the silu h1 plane still lands before its wave needs it.
        pre_abs = {}
        for f in range(fch):
            sl = slice(f * fw, (f + 1) * fw)
            nc.scalar.activation(ts_s[:, sl], xs_sb[:, sl], AF.Silu)
            if f == 2:
                t0 = fpool.tile([128, fw], DT.float16, name="t", tag="t")
                nc.scalar.activation(t0[:], xp_sb[0][:, 0:fw], AF.Abs,
                                     bias=bias_sb[:, 0:1], scale=2.5)
                pre_abs[0] = t0

        # spline chains (flat chunks; stage-major within each bc-half so the
        # DVE pipeline is not serialized behind one chunk's whole chain)
        for c in range(NSP):
            bias_ap = bias_sb[:, c:c + 1]
            for half in (0, 1):
                chunks = (2 * half, 2 * half + 1)
                tl = {}
                for f in chunks:
                    if c == 0 and f in pre_abs:
                        tl[f] = {"t": pre_abs[f]}
                        continue
                    sl = slice(f * fw, (f + 1) * fw)
                    t = fpool.tile([128, fw], DT.float16, name="t", tag="t")
                    nc.scalar.activation(t[:], xp_sb[c][:, sl], AF.Abs,
                                         bias=bias_ap, scale=2.5)
                    tl[f] = {"t": t}
                for f in chunks:
                    nr2 = fpool.tile([128, fw], DT.float16, name="nr2", tag="nr2")
                    nc.vector.tensor_scalar(nr2[:], tl[f]["t"][:], 2.0, 0.0,
                                            op0=AluOpType.subtract,
                                            op1=AluOpType.min)
                    nr1 = fpool.tile([128, fw], DT.float16, name="nr1", tag="nr1")
                    nc.vector.tensor_scalar(nr1[:], tl[f]["t"][:], 1.0, 0.0,
                                            op0=AluOpType.subtract,
                                            op1=AluOpType.min)
                    tl[f]["nr2"], tl[f]["nr1"] = nr2, nr1
                for f in chunks:
                    s2 = fpool.tile([128, fw], DT.float16, name="s2", tag="s2")
                    if S2_ACT[c][f]:
                        nc.scalar.activation(s2[:], tl[f]["nr2"][:], AF.Square)
                    else:
                        nc.vector.tensor_tensor(s2[:], tl[f]["nr2"][:],
                                                tl[f]["nr2"][:], op=AluOpType.mult)
                    tl[f]["s2"] = s2
                for f in chunks:
                    s1f = fpool.tile([128, fw], DT.float16, name="s1f", tag="s1f")
                    if S1F_ACT[c][f]:
                        nc.scalar.activation(s1f[:], tl[f]["nr1"][:], AF.Square,
                                             scale=2.0)
                    else:
                        nc.vector.scalar_tensor_tensor(
                            s1f[:], tl[f]["nr1"][:], 4.0, tl[f]["nr1"][:],
                            op0=AluOpType.mult, op1=AluOpType.mult)
                    tl[f]["s1f"] = s1f
                for f in chunks:
                    c2n = fpool.tile([128, fw], DT.float16, name="c2n", tag="c2n")
                    nc.vector.tensor_tensor(c2n[:], tl[f]["s2"][:],
                                            tl[f]["nr2"][:], op=AluOpType.mult)
                    tl[f]["c2n"] = c2n
                for f in chunks:
                    cn1 = fpool.tile([128, fw], DT.float16, name="cn1", tag="cn1")
                    nc.vector.tensor_tensor(cn1[:], tl[f]["s1f"][:],
                                            tl[f]["nr1"][:], op=AluOpType.mult)
                    tl[f]["cn1"] = cn1
                for f in chunks:
                    sl = slice(f * fw, (f + 1) * fw)
                    nc.vector.tensor_tensor(ts_t[c][:, sl], tl[f]["c2n"][:],
                                            tl[f]["cn1"][:], op=AluOpType.subtract)

        def emit_mm(g, seq, x, start=False, stop=False):
            kh, kw = g
            kind, c, j = seq
            if kind == "S":
                idx = kw * 6 + j
                src = ts_s
            else:
                idx = 18 + c * 36 + j * 3 + kw     # j-major pass tiles
                src = ts_t[c]
            lhsT = wq_sb[:, idx * 128:(idx + 1) * 128]
            h0 = kh + j
            ps = ps_tiles[g][:].rearrange("p (q b) -> p q b", b=BC)
            rhs = src[:].rearrange("p (x h b) -> p x h b", x=2, b=64)[
                :, x, h0:h0 + 37:12, :]
            out_ap = ps[:, :, 64 * x:64 * (x + 1)]
            nc.tensor.matmul(out_ap, lhsT, rhs, start=start, stop=stop)

        def drain(g):
            ob = opool.tile([128, 512], DT.float16)
            # adds the constant contribution of the removed w_pad 0/49 slots;
            # on DVE (idle, empty queue at the tail) so the final drain
            # dispatches right after its group's stop matmul.  Split in
            # halves so the first out-DMA overlaps the second half's drain.
            for h0, h1 in ((0, 256), (256, 512)):
                nc.vector.tensor_scalar(ob[:, h0:h1], ps_tiles[g][:, h0:h1],
                                        bias_sb[:, 4 + g[1]:5 + g[1]], None,
                                        op0=AluOpType.add)
                nc.sync.dma_start(out[g[0] * 3 + g[1]][:, h0:h1], ob[:, h0:h1])

        wave = groups[:8]
        silu_seqs = [("S", 0, j) for j in range(6)]
        pass_seqs = [[("P", c, j) for j in range(12)] for c in range(NSP)]

        def kworder(include_g8=False):
            gs = groups[:9] if include_g8 else groups[:8]
            return sorted(gs, key=lambda g: (g[1], g[0]))  # kw major, kh inner

        # per bc-half waves; kh-inner triples share lhsT
        for x in (0, 1):
            for j, s in enumerate(silu_seqs):
                for g in kworder():
                    emit_mm(g, s, x, start=(x == 0 and j == 0))
        # zero-weight fillers keep the PE (and its HAM clock state) busy
        # through the pass-0 plane wait; they add 0 to a live bank
        for _ in range(24):
            nc.tensor.matmul(ps_tiles[groups[0]][:, 0:256], zt[:],
                             ts_s[:, 0:256], start=False, stop=False)
        for c in (0, 1):
            for x in (0, 1):
                for s in pass_seqs[c]:
                    for g in kworder():
                        emit_mm(g, s, x)
        # pass 2: group 0 first, drain it, then g8's full run, then the rest
        for x in (0, 1):
            for i, s in enumerate(pass_seqs[2]):
                emit_mm(wave[0], s, x, stop=(x == 1 and i == 11))
        drain(wave[0])
        g8 = groups[8]
        ps_tiles[g8] = psum_pool.tile([128, 512], DT.float32, name="ps_22",
                                      tag="ps")
        g8_seqs = [(s, x) for s in silu_seqs + pass_seqs[0] + pass_seqs[1]
                   + pass_seqs[2] for x in (0, 1)]
        for i, (s, x) in enumerate(g8_seqs):
            emit_mm(g8, s, x, start=(i == 0), stop=(i == len(g8_seqs) - 1))
        drain(g8)
        for g in [g for g in kworder() if g != wave[0]]:
            for x in (0, 1):
                for i, s in enumerate(pass_seqs[2]):
                    emit_mm(g, s, x, stop=(x == 1 and i == 11))
            drain(g)
    nc.compile()
    return nc


def _prep_weights(base_weight, spline_weight, spline_scaler):
    # Wf[oc, func, jj]: func 0 = silu weights, 1+g = scaled spline / -6
    wf = np.empty((OUT_C, 9, 576), dtype=np.float64)
    wf[:, 0, :] = base_weight
    wf[:, 1:, :] = np.moveaxis(
        spline_weight.astype(np.float64)
        * spline_scaler.astype(np.float64)[..., None] / -6.0, -1, 1)
    w4 = wf.reshape(OUT_C, 9, 12, 48)
    wq = np.zeros((128, NTILE, OUT_C), dtype=np.float64)
    for kw in range(3):
        for jj in range(6):  # silu tiles
            idx = kw * 6 + jj
            for p in range(96):
                s, wp = p // 48, 1 + p % 48
                jw = wp - kw
                if 0 <= jw < 48:
                    wq[p, idx, :] = w4[:, 0, jj + 6 * s, jw]
    for c in range(NSP):
        for kw in range(3):
            for jh in range(12):
                idx = 18 + c * 36 + jh * 3 + kw   # j-major pass tiles
                for p in range(128):
                    flat = 128 * c + p
                    g, wp = flat // 48, 1 + flat % 48
                    jw = wp - kw
                    if 0 <= jw < 48:
                        wq[p, idx, :] = w4[:, 1 + g, jh, jw]
    wq = wq.reshape(128, NTILE * 128).astype(np.float16)

    bias = np.zeros((128, 8), dtype=np.float32)
    for c in range(NSP):
        for p in range(128):
            bias[p, c] = 3.5 - (128 * c + p) // 48
    # drain-time constant for removed w_pad 0 (kw=0) / 49 (kw=2) slots
    for g in range(8):
        bias[:, 4] += V0[g] * w4[:, 1 + g, :, 0].sum(axis=1)
        bias[:, 6] += V0[g] * w4[:, 1 + g, :, 47].sum(axis=1)
    return wq, bias


def _prep_x(x_slice):
    # x_slice: (B, CLOC, 48, 48) -> (xs [128, FREE], xp [128, NSP*FREE]) fp16
    # xs and xp[0] are bc-half-blocked: col = x*3200 + h*64 + b'
    # xp[1], xp[2] classic: col = h*128 + b
    plane = np.zeros((HP, HP, BC), dtype=np.float32)
    plane[1:49, 1:49, :] = np.ascontiguousarray(
        x_slice.transpose(3, 2, 0, 1)).reshape(48, 48, BC)
    flat = plane.reshape(HP, FREE)          # [w_pad, h*bc]
    sh6 = np.zeros_like(plane)              # h-shift by 6
    sh6[:, 0:44, :] = plane[:, 6:50, :]
    flat6 = sh6.reshape(HP, FREE)

    def blocked(a):  # [rows, h*bc] -> [rows, (x h b')]
        r = a.reshape(-1, HP, 2, 64)
        return np.ascontiguousarray(r.transpose(0, 2, 1, 3)).reshape(-1, FREE)

    xs = np.zeros((128, FREE), dtype=np.float16)
    xs[0:48] = blocked(flat[1:49])
    xs[48:96] = blocked(flat6[1:49])
    xp = np.empty((128, NSP * FREE), dtype=np.float16)
    for c in range(NSP):
        rows = [1 + (128 * c + p) % 48 for p in range(128)]
        xp[:, c * FREE:(c + 1) * FREE] = blocked(flat[rows])
    return xs, xp


def kernel(x, base_weight, spline_weight, spline_scaler):
    x = np.asarray(x, dtype=np.float32)
    wq, bias = _prep_weights(np.asarray(base_weight), np.asarray(spline_weight),
                             np.asarray(spline_scaler))
    nc = build_nc()
    in_maps = []
    for k in range(NCORES):
        xs, xp = _prep_x(x[:, k * CLOC:(k + 1) * CLOC])
        in_maps.append({"xs": xs, "xp": xp, "wq": wq, "bias": bias})
    res = run_bass_kernel_spmd(nc, in_maps, list(range(NCORES)), **RUN_KWARGS)
    global LAST_EXEC_NS
    LAST_EXEC_NS = res.exec_time_ns
    outs = [np.asarray(r["out"]) for r in res.results]

    full = np.empty((B, 2304, OUT_C), dtype=np.float32)
    for k in range(NCORES):
        dev = outs[k].astype(np.float32).reshape(3, 3, OUT_C, 4, B, CLOC)
        rows = dev.transpose(4, 5, 0, 1, 3, 2).reshape(B, 288, OUT_C)
        full[:, 288 * k:288 * (k + 1), :] = rows
    return full.reshape(B, 128, 2304).reshape(B, 128, 48, 48)
